# revision 1
# baseline (speedup 1.0000x reference)
"""Trainium2 Bass kernel for nn_BalancedHamiltonLayer.

The reference computes, per token-matrix X_n (32x32 view of each 1024-dim
token):  out_n = sum_r H_r @ X_n @ B_r^T  + bias, with H_r the 32x32 Hamilton
matrix of A_stack[r].  That is a fixed linear map on the flattened token:

    out[n, k*32+j] = sum_{s,i} W[s*32+i, k*32+j] * x[n, s*32+i]
    W[si, kj]      = sum_r H[r,k,s] * B[r,j,i]          (1024x1024, fp32)

so the whole layer is one dense (12288, 1024) @ (1024, 1024) matmul + bias.
The tiny factor stacks are folded on the host; x is sharded along the batch
axis across the 8 NeuronCores (data parallel, no collectives).

W's 4x4 grid of 256x256 blocks are +-copies of only FOUR unique matrices
M_q[(sr,i),(kr,j)] = sum_r A[r,q,kr,sr]*B[r,j,i] (quaternion structure), so
the host ships 2 MB of +-M instead of 4 MB of W, and every matmul rhs is a
(128, 256) view straight into that pack - no on-chip weight expansion.

Per core (1536 tokens, 12 blocks of 128), software-pipelined:
  natural-layout x DMA -> PE transpose (fp32r, via identity) into PSUM ->
  one DVE evacuation per block -> 24 fp32r matmuls (K=8x128; 8x N=512 where
  the quaternion signs allow slab-merging, 16x N=256 otherwise)
  accumulating in PSUM -> DVE bias-add -> DMA out (scalar-engine HWDGE).
Transposes are emitted one block ahead of the matmuls so the DVE evacuation
overlaps the previous block's matmuls instead of stalling the PE.

PSUM bank accumulation: each bank's k=0 matmul is a full-bank N=512 write
with start=True (clears the bank's has_written bits); later 256-column
slices overwrite-where-clear or accumulate per element.  fp32r (rounded fp32) streams at 1 cycle/row for N>=256 (4x
faster than fp32) at ~1.5e-4 relative error.

Self-loading 4-byte Matmults only fit ONE sync wait in the S3_LW ISA
struct, so the kernel keeps every Matmult at <=1 wait: PE warm-up
transposes (interleaved with block 0's matmuls) absorb the weight-pack DMA
deps, and all PSUM evacuations go through the vector engine so a single
DVE-sem wait on the first matmul of each block transitively covers every
WAR hazard.
"""

import numpy as np

B, T, D = 48, 256, 1024
N_CORES = 8
TOK = B * T                     # 12288 tokens
TOK_PER_CORE = TOK // N_CORES   # 1536
BLOCKS = TOK_PER_CORE // 128    # 12
KT = D // 128                   # 8 contraction tiles

# Quaternion block structure: W block (sb, kb) = SGN[kb][sb] * M[Q[kb][sb]]
Q_IDX = [[0, 1, 2, 3], [1, 0, 3, 2], [2, 3, 0, 1], [3, 2, 1, 0]]
SGN = [[1, -1, -1, -1], [1, 1, -1, 1], [1, 1, 1, -1], [1, -1, 1, 1]]

_cached_nc = None


def _build():
    import concourse.bacc as bacc
    import concourse.mybir as mybir
    import concourse.tile as tile

    F32R = mybir.dt.float32r
    F32 = mybir.dt.float32

    nc = bacc.Bacc("TRN2", target_bir_lowering=False)
    x_d = nc.dram_tensor("x", [TOK_PER_CORE, D], F32R, kind="ExternalInput")
    mp_d = nc.dram_tensor("mpack", [128, 2, 4, 256], F32R, kind="ExternalInput")
    b_d = nc.dram_tensor("biasb", [1, D], F32, kind="ExternalInput")
    o_d = nc.dram_tensor("out", [TOK_PER_CORE, D], F32, kind="ExternalOutput")

    with tile.TileContext(nc) as tc:
        with (
            tc.tile_pool(name="consts", bufs=1) as consts,
            tc.tile_pool(name="xin", bufs=6) as xin_pool,
            tc.tile_pool(name="xt", bufs=6) as xt_pool,
            tc.tile_pool(name="outp", bufs=BLOCKS) as out_pool,
            tc.tile_pool(name="psum_t", bufs=2, space="PSUM") as psum_t,
            tc.tile_pool(name="psum_mm", bufs=4, space="PSUM") as psum_mm,
        ):
            # identity built on-chip: gpsimd writes f32, DVE cast-copies to
            # f32r (the canonical "round to fp32r" producer) - no DMA.
            from concourse.masks import make_identity

            identity_f32 = consts.tile([128, 128], F32)
            make_identity(nc, identity_f32)
            identity = consts.tile([128, 128], F32R)
            nc.vector.tensor_copy(out=identity[:], in_=identity_f32[:])

            m_sb = consts.tile([128, 2, 2, 4, 256], F32R)
            # Prefetch the first x blocks ahead of the weight pack so the PE
            # has transpose work while the weights stream in.
            early_x = {}
            for blk in range(2):
                x_sb = xin_pool.tile([128, D], F32R, tag="x_sb", name=f"x_sb_{blk}")
                if blk == 0:
                    nc.sync.dma_start(x_sb[:, 0:512], x_d[0:128, 0:512])
                    nc.sync.dma_start(x_sb[:, 512:D], x_d[0:128, 512:D])
                else:
                    nc.sync.dma_start(x_sb[:], x_d[blk * 128 : (blk + 1) * 128, :])
                early_x[blk] = x_sb


            # Weight pack DMAs: +M only (1 MB); -M is produced on the DVE
            # during block 0 (cheaper than 1 MB more of front-loaded DMA).
            for h in (0, 1):
                nc.sync.dma_start(m_sb[:, 0, h], mp_d[:, h])

            # bias: 4 KB DMA + on-chip broadcast (gpsimd is otherwise idle)
            bias_row = consts.tile([1, D], F32)
            nc.sync.dma_start(bias_row[:], b_d[:])
            bias_sb = consts.tile([128, D], F32)
            nc.gpsimd.partition_broadcast(bias_sb[:], bias_row[:])

            def rhs_view(k, kb):
                sb, half = k // 2, k % 2
                sidx = 0 if SGN[kb][sb] > 0 else 1
                return m_sb[:, sidx, half, Q_IDX[kb][sb], :]

            def block_mm_items(k):
                """(n, c, rhs) triples covering kb 0..3 of chunk k.  For
                sb=0 and sb=2 the kb-pairs are adjacent same-sign q-slabs in
                the pack, so they merge into N=512 matmuls (c=None) - same
                PE cycles, 25% fewer self-weight-loads."""
                sb, half = k // 2, k % 2
                if sb == 0:
                    return [
                        (0, None, m_sb[:, 0, half, 0:2, :]),
                        (1, None, m_sb[:, 0, half, 2:4, :]),
                    ]
                if sb == 2:
                    return [
                        (0, None, m_sb[:, 1, half, 2:4, :]),
                        (1, None, m_sb[:, 0, half, 0:2, :]),
                    ]
                return [
                    (kb // 2, kb % 2, rhs_view(k, kb)) for kb in range(4)
                ]

            # PE warm-up absorbs the identity producer dep so the first
            # transpose carries only its x-DMA wait (ONE wait per Matmult).
            warm_a = psum_mm.tile([128, 128], F32R, tag="mm_ps")
            nc.tensor.transpose(warm_a[:], identity[:], identity[:])
            # dummy matmuls fill the initial DMA wait so the HAM clock-gate
            # is open when real matmuls start (transposes don't count as
            # PE-busy for HAM)
            for _w in range(12):
                warm_m = psum_mm.tile([128, 128], F32, tag="mm_ps", name=f"warm_m{_w}")
                nc.tensor.matmul(warm_m[:], identity[:], identity[:], start=True, stop=True)

            # Warm-up transposes that absorb each weight-pack DMA wait,
            # emitted just before the block-0 matmul that first needs it.
            warm_before = {0: [(0, 0)], 1: [(0, 1)], 2: [(1, 0)], 3: [(1, 1)]}

            xt_tiles = {}
            # Software pipeline: stage A (DMA + transpose + evacuate) runs
            # one block ahead of stage B (matmuls + bias-add + store).
            for blk in range(BLOCKS + 1):
                if blk < BLOCKS:
                    rows = slice(blk * 128, (blk + 1) * 128)
                    if blk in early_x:
                        x_sb = early_x.pop(blk)
                    else:
                        x_sb = xin_pool.tile([128, D], F32R, tag="x_sb")
                        nc.sync.dma_start(x_sb[:], x_d[rows, :])
                    xt_ps = psum_t.tile([128, D], F32R, tag="xt_ps")
                    for k in range(KT):
                        nc.tensor.transpose(
                            xt_ps[:, k * 128 : (k + 1) * 128],
                            x_sb[:, k * 128 : (k + 1) * 128],
                            identity[:],
                        )
                    xt_sb = xt_pool.tile([128, D], F32R, tag="xt_sb")
                    if blk == 0:
                        nc.vector.tensor_copy(out=xt_sb[:, 0:512], in_=xt_ps[:, 0:512])
                        nc.vector.tensor_copy(out=xt_sb[:, 512:D], in_=xt_ps[:, 512:D])
                    else:
                        nc.vector.tensor_copy(out=xt_sb[:], in_=xt_ps[:])
                    xt_tiles[blk] = xt_sb
                    if blk == 0:
                        # negate the weight pack halves as they arrive
                        for h in (0, 1):
                            nc.vector.tensor_scalar_mul(
                                m_sb[:, 1, h], m_sb[:, 0, h], -1.0
                            )

                if blk >= 1:
                    mblk = blk - 1
                    rows = slice(mblk * 128, (mblk + 1) * 128)
                    xt_sb = xt_tiles.pop(mblk)
                    out_sb = out_pool.tile([128, D], F32, tag="out_sb")
                    mm_ps = [
                        psum_mm.tile(
                            [128, 512], F32, tag="mm_ps", name=f"mm_ps_{mblk}_{n}"
                        )
                        for n in range(2)
                    ]
                    # k-outer: each stationary xt slice loads once for all
                    # of its output slabs.
                    last = mblk == BLOCKS - 1
                    items = [
                        (k, n, c, rhs)
                        for k in range(KT)
                        for (n, c, rhs) in block_mm_items(k)
                    ]
                    if last:
                        # bank-outer: close bank 0 halfway through so its
                        # bias-add + store overlap bank 1's matmuls
                        items.sort(key=lambda t: t[1])
                    seen_k = set()
                    for k, n, c, rhs in items:
                        if mblk == 0 and k not in seen_k:
                            seen_k.add(k)
                            for s, h in warm_before.get(k, []):
                                warm_k = psum_mm.tile(
                                    [128, 128], F32R, tag="mm_ps", name=f"warm_{s}{h}"
                                )
                                nc.tensor.transpose(
                                    warm_k[:], m_sb[:, s, h, 0, 0:128], identity[:]
                                )
                        dst = (
                            mm_ps[n][:]
                            if c is None
                            else mm_ps[n][:, c * 256 : (c + 1) * 256]
                        )
                        nc.tensor.matmul(
                            dst,
                            xt_sb[:, k * 128 : (k + 1) * 128],
                            rhs,
                            start=(k == 0),
                            stop=(k == KT - 1 and c == 1),
                            skip_group_check=True,
                        )
                    if mblk == 0:
                        # DVE warm-up observes the bias DMA queue before the
                        # first add so the add itself carries one wait.
                        warm_v = consts.tile([128, 1], F32)
                        nc.vector.tensor_copy(out=warm_v[:], in_=bias_sb[:, 0:1])
                    for n in range(2):
                        nc.vector.tensor_add(
                            out=out_sb[:, n * 512 : (n + 1) * 512],
                            in0=mm_ps[n][:],
                            in1=bias_sb[:, n * 512 : (n + 1) * 512],
                        )
                        eng = nc.sync if (last and n == 1) else nc.scalar
                        eng.dma_start(
                            o_d[rows, n * 512 : (n + 1) * 512],
                            out_sb[:, n * 512 : (n + 1) * 512],
                        )
    nc.compile()
    return nc


def kernel(x, A_stack, B_stack, bias):
    from concourse.bass_utils import run_bass_kernel_spmd

    global _cached_nc
    x = np.ascontiguousarray(np.asarray(x, dtype=np.float32))
    A_stack = np.asarray(A_stack, dtype=np.float32)
    B_stack = np.asarray(B_stack, dtype=np.float32)
    bias = np.asarray(bias, dtype=np.float32)

    # M_q[(sr,i),(kr,j)] = sum_r A[r,q,kr,sr] * B[r,j,i]; W block (sb,kb)
    # = SGN[kb][sb] * M[Q[kb][sb]] reproduces W[si,kj] = sum_r H B.
    M = np.einsum("rqks,rji->qsikj", A_stack, B_stack).reshape(4, 256, 256)
    mpack = np.empty((128, 2, 4, 256), dtype=np.float32)
    for h in range(2):
        mpack[:, h] = np.moveaxis(M[:, h * 128 : (h + 1) * 128, :], 0, 1)
    mpack = np.ascontiguousarray(mpack)

    bias_b = np.ascontiguousarray(bias[None, :])

    shards = x.reshape(N_CORES, TOK_PER_CORE, D)
    if _cached_nc is None:
        _cached_nc = _build()
    in_maps = [
        {"x": shards[c], "mpack": mpack, "biasb": bias_b}
        for c in range(N_CORES)
    ]
    try:
        res = run_bass_kernel_spmd(
            _cached_nc, in_maps, core_ids=list(range(N_CORES)), trace=False
        )
    except Exception:
        # axon terminals occasionally throw a transient device error
        # (NRT_EXEC_UNIT_UNRECOVERABLE) that recovers on retry
        res = run_bass_kernel_spmd(
            _cached_nc, in_maps, core_ids=list(range(N_CORES)), trace=False
        )
    out = np.concatenate([r["out"] for r in res.results], axis=0)
    return out.reshape(B, T, D)



# revision 2
# speedup vs baseline: 1.0130x; 1.0130x over previous
"""Trainium2 Bass kernel for nn_BalancedHamiltonLayer — quaternion rank-8.

out_n = sum_r H_r @ X_n @ B_r^T + bias collapses to a dense (1536,1024) @
(1024,1024) matmul per core whose 4x4 grid of 256x256 blocks is the left-
multiplication table of a quaternion M = (M0,M1,M2,M3):

    out_k = sum_s SGN[k][s] * x_s @ M_{k XOR s}

A rank-8 bilinear scheme computes this with 8 products p_t = c_t @ N_t
(c_t = +-1 combination of the four x sub-blocks, N_t = fixed combination of
the M_q, folded on the host) instead of 16:

    c0=x1-x3  c1=x1+x3  c2=x0+x3  c3=x1-x2  c4=x0-x3  c5=x1+x2  c6=x0-x2
    c7=x0+x2
    N0=(M0-M1-M2+M3)/2  N1=-(M0+M1+M2+M3)/2  N2=M0  N3=M2  N4=M1  N5=-M3
    N6=(-M0-M1+M2+M3)/2 N7=(M0-M1+M2-M3)/2
    O0=p0+p1+p2+p3  O1=p0-p1+p4+p5  O2=p4-p5+p6+p7  O3=p2-p3+p6-p7

All I/O is fp16 (the 2e-2 gate leaves ~20x headroom): the host ships the
eight x-combos pre-transposed, the device returns fp16, the host upcasts
and adds bias.

Per 128-token block (PSUM tile [128,2048], regions [O0|O3|O1|O2|m0|m1|m6|m7]):
  p2,p3 (resp. p4,p5) each stream ONCE as an N=512 matmul into the adjacent
  accumulator pair [O0|O3] ([O1|O2]) with per-half signs folded into the
  duplicated/negated weight pack -> their second use costs no vector work;
  p0,p1,p6,p7 materialize, ACT evacuates them pairwise ([128,512] copies)
  to fp16, DVE does the four psum+fp16 adds, Pool/DVE finish in fp16:
    out0 = (O0 + m0) + m1   out1 = (O1 + m0) - m1
    out2 = (O2 + m6) + m7   out3 = (O3 + m6) - m7
  16 matmuls/block (engine 6144 cyc).  In-DMA on the SP queue only (out on
  ACT) so prefetches never sit behind post-dependent stores.

Matmults carry at most ONE sync wait: a zero-cost standalone ldweights
absorbs each block's x-DMA wait (3 more in block 0 for the staged weight
DMAs); each PSUM region group's first matmul carries only its single WAR
wait, with all stream-target consumers on DVE and all materialized-product
consumers on ACT.
"""

import numpy as np

B, T, D = 48, 256, 1024
N_CORES = 8
TOK = B * T
TOK_PER_CORE = TOK // N_CORES   # 1536
BLOCKS = TOK_PER_CORE // 128    # 12
NT = 14                         # weight tiles incl. duplicated/negated copies

_cached_nc = None


def _build():
    import concourse.bacc as bacc
    import concourse.mybir as mybir
    import concourse.tile as tile

    F16 = mybir.dt.float16
    F32 = mybir.dt.float32

    nc = bacc.Bacc("TRN2", target_bir_lowering=False)
    x_d = nc.dram_tensor("xin", [BLOCKS, 128, 8, 256], F16, kind="ExternalInput")
    mp_d = nc.dram_tensor("mpack", [128, NT, 2, 256], F16, kind="ExternalInput")
    o_d = nc.dram_tensor("out", [BLOCKS, 128, D], F16, kind="ExternalOutput")

    with tile.TileContext(nc) as tc:
        with (
            tc.tile_pool(name="consts", bufs=1) as consts,
            tc.tile_pool(name="xin", bufs=8) as xin_pool,
            tc.tile_pool(name="msb", bufs=4) as msb_pool,
            tc.tile_pool(name="tch", bufs=4) as tch_pool,
            tc.tile_pool(name="outp", bufs=5) as out_pool,
            tc.tile_pool(name="psum", bufs=2, space="PSUM") as psum_pool,
        ):
            m_sb = consts.tile([128, NT, 2, 256], F16)

            xin = {}
            pending_out = []

            def fetch(b):
                # inputs alternate between the SP and ACT hwdge queues so
                # neither serializes the whole input stream
                t = xin_pool.tile([128, 8, 256], F16, tag="xin", name=f"xin{b}")
                eng = nc.sync if b % 2 == 0 else nc.scalar
                eng.dma_start(t[:], x_d[b])
                xin[b] = t

            # issue order: W1 (materialized-product tiles) early, stream
            # tiles (W2a: [O0|O3], W2b: [O1|O2]) interleaved with the first
            # x fetches
            nc.sync.dma_start(m_sb[:, 0:4], mp_d[:, 0:4])
            fetch(0)
            nc.scalar.dma_start(m_sb[:, 4:8], mp_d[:, 4:8])
            nc.scalar.dma_start(m_sb[:, 8:12], mp_d[:, 8:12])
            fetch(1)
            fetch(2)
            fetch(3)

            for blk in range(BLOCKS):
                if blk + 4 < BLOCKS:
                    fetch(blk + 4)
                if blk == 5:
                    # tiles for the all-stream final block ([N1|N1n],
                    # [N7|N7n] second halves) — needed only at block 11
                    nc.scalar.dma_start(m_sb[:, 12:14], mp_d[:, 12:14])
                xb = xin.pop(blk)

                ps = psum_pool.tile([128, 2048], F32, tag="ps")
                O0, O3 = ps[:, 0:256], ps[:, 256:512]
                O1, O2 = ps[:, 512:768], ps[:, 768:1024]
                m01 = ps[:, 1024:1536]
                m67 = ps[:, 1536:2048]

                # absorb the x-DMA wait on a zero-cost weight load so every
                # real Matmult below carries at most one (WAR) wait.  In
                # block 0 the three weight-stage absorbers are interleaved
                # right before the first matmul of each stage (an absorber
                # placed earlier would gate the whole in-order PE queue).
                nc.tensor.ldweights(xb[:, 0, 0:128])
                if blk == 0:
                    nc.tensor.ldweights(m_sb[:, 0, 0, 0:128])

                def mm(dst, plane, tslice, start, stop):
                    for k in range(2):
                        nc.tensor.matmul(
                            dst,
                            xb[:, plane, k * 128 : (k + 1) * 128],
                            m_sb[:, tslice, k, :],
                            start=start and k == 0,
                            stop=stop and k == 1,
                            skip_group_check=True,
                        )

                last = blk == BLOCKS - 1
                out_sb = out_pool.tile([128, D], F16, tag="out_sb")
                if not last:
                    # materialized products (N=256)
                    mm(ps[:, 1024:1280], 0, 0, True, True)   # p0 = c0 @ N0
                    mm(ps[:, 1280:1536], 1, 1, True, True)   # p1 = c1 @ N1
                    mm(ps[:, 1536:1792], 6, 2, True, True)   # p6 = c6 @ N6
                    mm(ps[:, 1792:2048], 7, 3, True, True)   # p7 = c7 @ N7
                    # dual-destination streams (N=512): [O0|O3] and [O1|O2]
                    if blk == 0:
                        nc.tensor.ldweights(m_sb[:, 4, 0, 0:128])
                    mm(ps[:, 0:512], 2, slice(4, 6), True, False)    # [+p2|+p2]
                    mm(ps[:, 512:1024], 4, slice(6, 8), True, False)   # [+p4|+p4]
                    if blk == 0:
                        nc.tensor.ldweights(m_sb[:, 8, 0, 0:128])
                    mm(ps[:, 0:512], 3, slice(8, 10), False, True)   # [+p3|-p3]
                    mm(ps[:, 512:1024], 5, slice(10, 12), False, True)  # [+p5|-p5]
                else:
                    # ALL-STREAM final block: every product accumulates
                    # directly in PSUM (no vector chain at the tail), and
                    # the [O1|O2] half completes first so its evacuation +
                    # store overlap the [O0|O3] matmuls
                    mm(ps[:, 512:1024], 4, slice(6, 8), True, False)     # [+p4|+p4]
                    mm(ps[:, 512:1024], 5, slice(10, 12), False, False)  # [+p5|-p5]
                    mm(ps[:, 512:768], 0, 0, False, False)    # +p0 -> O1
                    mm(ps[:, 512:768], 1, 12, False, False)   # -p1 -> O1 (N1n)
                    mm(ps[:, 768:1024], 6, 2, False, False)   # +p6 -> O2
                    mm(ps[:, 768:1024], 7, 3, False, True)    # +p7 -> O2
                    nc.scalar.copy(out_sb[:, 256:768], ps[:, 512:1024])
                    nc.gpsimd.dma_start(o_d[blk, :, 256:768], out_sb[:, 256:768])
                    mm(ps[:, 0:512], 2, slice(4, 6), True, False)      # [+p2|+p2]
                    mm(ps[:, 0:512], 3, slice(8, 10), False, False)    # [+p3|-p3]
                    mm(ps[:, 0:256], 0, 0, False, False)      # +p0 -> O0
                    mm(ps[:, 0:256], 1, 1, False, False)      # +p1 -> O0
                    mm(ps[:, 256:512], 6, 2, False, False)    # +p6 -> O3
                    mm(ps[:, 256:512], 7, 13, False, True)    # -p7 -> O3 (N7n)

                if not last:
                    msb = msb_pool.tile([128, 2, 2, 256], F16, tag="msb")
                    nc.scalar.copy(msb[:, 0], m01)
                    nc.scalar.copy(msb[:, 1], m67)

                # issue a TWO-block-old store now (after this block's
                # evacs): its producers are long done, so the DMA carries
                # no blocking wait; Pool (SWDGE) has no other engine work
                if len(pending_out) >= 2:
                    dst, src = pending_out.pop(0)
                    nc.gpsimd.dma_start(dst, src)

                if not last:
                    t03 = tch_pool.tile([128, 512], F16, tag="t03")
                    t12 = tch_pool.tile([128, 512], F16, tag="t12")
                    # DVE: two 512-wide psum + [m0|m6] adds, then four fast
                    # (2x fp16) finishes — Pool only issues stores
                    nc.vector.tensor_add(t03[:], ps[:, 0:512], msb[:, :, 0, :])
                    nc.vector.tensor_add(t12[:], ps[:, 512:1024], msb[:, :, 0, :])
                    nc.vector.tensor_add(out_sb[:, 0:256], t03[:, 0:256], msb[:, 0, 1])
                    nc.vector.tensor_sub(out_sb[:, 256:512], t12[:, 0:256], msb[:, 0, 1])
                    nc.vector.tensor_add(out_sb[:, 512:768], t12[:, 256:512], msb[:, 1, 1])
                    nc.vector.tensor_sub(out_sb[:, 768:1024], t03[:, 256:512], msb[:, 1, 1])
                else:
                    # remaining evacuations ([O1|O2] already stored above):
                    # DVE and ACT in parallel, stores on separate queues
                    nc.vector.tensor_copy(out=out_sb[:, 0:256], in_=ps[:, 0:256])
                    nc.scalar.copy(out_sb[:, 768:1024], ps[:, 256:512])
                    for dst, src in pending_out:
                        nc.sync.dma_start(dst, src)
                    pending_out.clear()
                    nc.scalar.dma_start(o_d[blk, :, 0:256], out_sb[:, 0:256])
                    nc.gpsimd.dma_start(o_d[blk, :, 768:1024], out_sb[:, 768:1024])

                if not last:
                    pending_out.append((o_d[blk], out_sb[:]))
    nc.compile()
    return nc


def _pack_weights(A_stack, B_stack):
    # M_q[(sr,i),(kr,j)] = sum_r A[r,q,kr,sr] * B[r,j,i]
    M = np.einsum("rqks,rji->qsikj", A_stack, B_stack).reshape(4, 256, 256)
    N = np.stack([
        (M[0] - M[1] - M[2] + M[3]) * 0.5,    # N0
        -(M[0] + M[1] + M[2] + M[3]) * 0.5,   # N1
        (-M[0] - M[1] + M[2] + M[3]) * 0.5,   # N6
        (M[0] - M[1] + M[2] - M[3]) * 0.5,    # N7
        M[0], M[0],                            # [N2|N2]   -> [O0|O3]
        M[1], M[1],                            # [N4|N4]   -> [O1|O2]
        M[2], -M[2],                           # [N3|N3n]  -> [O0|O3]
        -M[3], M[3],                           # [N5|N5n]  -> [O1|O2]
        (M[0] + M[1] + M[2] + M[3]) * 0.5,     # N1n = -N1 (final block)
        -(M[0] - M[1] + M[2] - M[3]) * 0.5,    # N7n = -N7 (final block)
    ])  # [NT, 256, 256]
    mpack = N.reshape(NT, 2, 128, 256).transpose(2, 0, 1, 3)
    return np.ascontiguousarray(mpack.astype(np.float16))


def _pack_x(x):
    xf = np.asarray(x, dtype=np.float32).reshape(N_CORES, TOK_PER_CORE, D)
    xs = [xf[..., s * 256 : (s + 1) * 256] for s in range(4)]
    planes = np.stack(
        [
            xs[1] - xs[3], xs[1] + xs[3], xs[0] + xs[3], xs[1] - xs[2],
            xs[0] - xs[3], xs[1] + xs[2], xs[0] - xs[2], xs[0] + xs[2],
        ],
        axis=1,
    ).astype(np.float16)  # [core, 8, 1536, 256]
    # xin[core, b, p, t, k*128+n] = planes[core, t, b*128+n, k*128+p]
    pr = planes.reshape(N_CORES, 8, BLOCKS, 128, 2, 128)
    xin = pr.transpose(0, 2, 5, 1, 4, 3).reshape(N_CORES, BLOCKS, 128, 8, 256)
    return np.ascontiguousarray(xin)


def kernel(x, A_stack, B_stack, bias):
    from concourse.bass_utils import run_bass_kernel_spmd

    global _cached_nc
    A_stack = np.asarray(A_stack, dtype=np.float32)
    B_stack = np.asarray(B_stack, dtype=np.float32)
    bias = np.asarray(bias, dtype=np.float32)

    mpack = _pack_weights(A_stack, B_stack)
    xin = _pack_x(x)

    if _cached_nc is None:
        _cached_nc = _build()
    in_maps = [
        {"xin": xin[c], "mpack": mpack} for c in range(N_CORES)
    ]
    try:
        res = run_bass_kernel_spmd(
            _cached_nc, in_maps, core_ids=list(range(N_CORES)), trace=False
        )
    except Exception:
        # axon terminals occasionally throw a transient device error
        res = run_bass_kernel_spmd(
            _cached_nc, in_maps, core_ids=list(range(N_CORES)), trace=False
        )
    out = np.stack([r["out"] for r in res.results], axis=0)  # [8,12,128,1024] f16
    out = out.reshape(TOK, D).astype(np.float32) + bias
    return out.reshape(B, T, D)


# revision 3
# speedup vs baseline: 1.0158x; 1.0028x over previous
"""Trainium2 Bass kernel for nn_BalancedHamiltonLayer — quaternion rank-8.

out_n = sum_r H_r @ X_n @ B_r^T + bias collapses to a dense (1536,1024) @
(1024,1024) matmul per core whose 4x4 grid of 256x256 blocks is the left-
multiplication table of a quaternion M = (M0,M1,M2,M3):

    out_k = sum_s SGN[k][s] * x_s @ M_{k XOR s}

A rank-8 bilinear scheme computes this with 8 products p_t = c_t @ N_t
(c_t = +-1 combination of the four x sub-blocks, N_t = fixed combination of
the M_q, folded on the host) instead of 16:

    c0=x1-x3  c1=x1+x3  c2=x0+x3  c3=x1-x2  c4=x0-x3  c5=x1+x2  c6=x0-x2
    c7=x0+x2
    N0=(M0-M1-M2+M3)/2  N1=-(M0+M1+M2+M3)/2  N2=M0  N3=M2  N4=M1  N5=-M3
    N6=(-M0-M1+M2+M3)/2 N7=(M0-M1+M2-M3)/2
    O0=p0+p1+p2+p3  O1=p0-p1+p4+p5  O2=p4-p5+p6+p7  O3=p2-p3+p6-p7

All I/O is fp16 (the 2e-2 gate leaves ~20x headroom): the host ships the
eight x-combos pre-transposed, the device returns fp16, the host upcasts
and adds bias.

Per 128-token block (PSUM tile [128,2048], regions [O0|O3|O1|O2|m0|m1|m6|m7]):
  p2,p3 (resp. p4,p5) each stream ONCE as an N=512 matmul into the adjacent
  accumulator pair [O0|O3] ([O1|O2]) with per-half signs folded into the
  duplicated/negated weight pack -> their second use costs no vector work;
  p0,p1,p6,p7 materialize, ACT evacuates them pairwise ([128,512] copies)
  to fp16, DVE does the four psum+fp16 adds, Pool/DVE finish in fp16:
    out0 = (O0 + m0) + m1   out1 = (O1 + m0) - m1
    out2 = (O2 + m6) + m7   out3 = (O3 + m6) - m7
  16 matmuls/block (engine 6144 cyc).  In-DMA on the SP queue only (out on
  ACT) so prefetches never sit behind post-dependent stores.

Matmults carry at most ONE sync wait: a zero-cost standalone ldweights
absorbs each block's x-DMA wait (3 more in block 0 for the staged weight
DMAs); each PSUM region group's first matmul carries only its single WAR
wait, with all stream-target consumers on DVE and all materialized-product
consumers on ACT.
"""

import numpy as np

B, T, D = 48, 256, 1024
N_CORES = 8
TOK = B * T
TOK_PER_CORE = TOK // N_CORES   # 1536
BLOCKS = TOK_PER_CORE // 128    # 12
NT = 14                         # weight tiles incl. duplicated/negated copies

_cached_nc = None


def _build():
    import concourse.bacc as bacc
    import concourse.mybir as mybir
    import concourse.tile as tile

    F16 = mybir.dt.float16
    F32 = mybir.dt.float32

    nc = bacc.Bacc("TRN2", target_bir_lowering=False)
    x_d = nc.dram_tensor("xin", [BLOCKS, 128, 8, 256], F16, kind="ExternalInput")
    mp_d = nc.dram_tensor("mpack", [128, NT, 2, 256], F16, kind="ExternalInput")
    o_d = nc.dram_tensor("out", [BLOCKS, 128, D], F16, kind="ExternalOutput")

    with tile.TileContext(nc) as tc:
        with (
            tc.tile_pool(name="consts", bufs=1) as consts,
            tc.tile_pool(name="xin", bufs=8) as xin_pool,
            tc.tile_pool(name="msb", bufs=4) as msb_pool,
            tc.tile_pool(name="tch", bufs=4) as tch_pool,
            tc.tile_pool(name="outp", bufs=5) as out_pool,
            tc.tile_pool(name="psum", bufs=2, space="PSUM") as psum_pool,
        ):
            m_sb = consts.tile([128, NT, 2, 256], F16)

            xin = {}
            pending_out = []

            def fetch(b):
                # inputs alternate between the SP and ACT hwdge queues so
                # neither serializes the whole input stream
                t = xin_pool.tile([128, 8, 256], F16, tag="xin", name=f"xin{b}")
                eng = nc.sync if b % 2 == 0 else nc.scalar
                eng.dma_start(t[:], x_d[b])
                xin[b] = t

            # issue order: W1 (materialized-product tiles) early, stream
            # tiles (W2a: [O0|O3], W2b: [O1|O2]) interleaved with the first
            # x fetches
            nc.sync.dma_start(m_sb[:, 0:4], mp_d[:, 0:4])
            fetch(0)
            nc.scalar.dma_start(m_sb[:, 4:8], mp_d[:, 4:8])
            nc.scalar.dma_start(m_sb[:, 8:12], mp_d[:, 8:12])
            fetch(1)
            fetch(2)
            fetch(3)

            # warm-up: keep the PE continuously busy through the prologue
            # DMAs so the p-state ramp completes before real matmuls start
            wtile = consts.tile([128, 128], F16)
            nc.vector.memset(wtile[:], 0.25)
            ps_warm = psum_pool.tile([128, 2048], F32, tag="ps", name="ps_warm")
            for _w in range(40):
                nc.tensor.matmul(
                    ps_warm[:, 0:128], wtile[:], wtile[:],
                    start=True, stop=True, skip_group_check=True,
                )

            for blk in range(BLOCKS):
                if blk + 4 < BLOCKS:
                    fetch(blk + 4)
                if blk == 5:
                    # tiles for the all-stream final block ([N1|N1n],
                    # [N7|N7n] second halves) — needed only at block 11
                    nc.scalar.dma_start(m_sb[:, 12:14], mp_d[:, 12:14])
                xb = xin.pop(blk)

                ps = psum_pool.tile([128, 2048], F32, tag="ps")
                O0, O3 = ps[:, 0:256], ps[:, 256:512]
                O1, O2 = ps[:, 512:768], ps[:, 768:1024]
                m01 = ps[:, 1024:1536]
                m67 = ps[:, 1536:2048]

                # absorb the x-DMA wait on a zero-cost weight load so every
                # real Matmult below carries at most one (WAR) wait.  In
                # block 0 the three weight-stage absorbers are interleaved
                # right before the first matmul of each stage (an absorber
                # placed earlier would gate the whole in-order PE queue).
                nc.tensor.ldweights(xb[:, 0, 0:128])
                if blk == 0:
                    nc.tensor.ldweights(m_sb[:, 0, 0, 0:128])

                def mm(dst, plane, tslice, start, stop):
                    for k in range(2):
                        nc.tensor.matmul(
                            dst,
                            xb[:, plane, k * 128 : (k + 1) * 128],
                            m_sb[:, tslice, k, :],
                            start=start and k == 0,
                            stop=stop and k == 1,
                            skip_group_check=True,
                        )

                last = blk == BLOCKS - 1
                out_sb = out_pool.tile([128, D], F16, tag="out_sb")
                if not last:
                    # materialized products (N=256)
                    mm(ps[:, 1024:1280], 0, 0, True, True)   # p0 = c0 @ N0
                    mm(ps[:, 1280:1536], 1, 1, True, True)   # p1 = c1 @ N1
                    mm(ps[:, 1536:1792], 6, 2, True, True)   # p6 = c6 @ N6
                    mm(ps[:, 1792:2048], 7, 3, True, True)   # p7 = c7 @ N7
                    # dual-destination streams (N=512): [O0|O3] and [O1|O2]
                    if blk == 0:
                        for _w in range(8):
                            nc.tensor.matmul(ps_warm[:, 0:128], wtile[:], wtile[:], start=True, stop=True, skip_group_check=True)
                        nc.tensor.ldweights(m_sb[:, 4, 0, 0:128])
                    mm(ps[:, 0:512], 2, slice(4, 6), True, False)    # [+p2|+p2]
                    mm(ps[:, 512:1024], 4, slice(6, 8), True, False)   # [+p4|+p4]
                    if blk == 0:
                        for _w in range(6):
                            nc.tensor.matmul(ps_warm[:, 0:128], wtile[:], wtile[:], start=True, stop=True, skip_group_check=True)
                        nc.tensor.ldweights(m_sb[:, 8, 0, 0:128])
                    mm(ps[:, 0:512], 3, slice(8, 10), False, True)   # [+p3|-p3]
                    mm(ps[:, 512:1024], 5, slice(10, 12), False, True)  # [+p5|-p5]
                else:
                    # ALL-STREAM final block: every product accumulates
                    # directly in PSUM (no vector chain at the tail), and
                    # the [O1|O2] half completes first so its evacuation +
                    # store overlap the [O0|O3] matmuls
                    mm(ps[:, 512:1024], 4, slice(6, 8), True, False)     # [+p4|+p4]
                    mm(ps[:, 512:1024], 5, slice(10, 12), False, False)  # [+p5|-p5]
                    mm(ps[:, 512:768], 0, 0, False, False)    # +p0 -> O1
                    mm(ps[:, 512:768], 1, 12, False, False)   # -p1 -> O1 (N1n)
                    mm(ps[:, 768:1024], 6, 2, False, False)   # +p6 -> O2
                    mm(ps[:, 768:1024], 7, 3, False, True)    # +p7 -> O2
                    nc.scalar.copy(out_sb[:, 256:768], ps[:, 512:1024])
                    nc.gpsimd.dma_start(o_d[blk, :, 256:768], out_sb[:, 256:768])
                    mm(ps[:, 0:512], 2, slice(4, 6), True, False)      # [+p2|+p2]
                    mm(ps[:, 0:512], 3, slice(8, 10), False, False)    # [+p3|-p3]
                    mm(ps[:, 0:256], 0, 0, False, False)      # +p0 -> O0
                    mm(ps[:, 0:256], 1, 1, False, False)      # +p1 -> O0
                    mm(ps[:, 256:512], 6, 2, False, False)    # +p6 -> O3
                    mm(ps[:, 256:512], 7, 13, False, True)    # -p7 -> O3 (N7n)

                if not last:
                    msb = msb_pool.tile([128, 2, 2, 256], F16, tag="msb")
                    nc.scalar.copy(msb[:, 0], m01)
                    nc.scalar.copy(msb[:, 1], m67)

                # issue a TWO-block-old store now (after this block's
                # evacs): its producers are long done, so the DMA carries
                # no blocking wait; Pool (SWDGE) has no other engine work
                if len(pending_out) >= 2:
                    dst, src = pending_out.pop(0)
                    nc.gpsimd.dma_start(dst, src)

                if not last:
                    t03 = tch_pool.tile([128, 512], F16, tag="t03")
                    t12 = tch_pool.tile([128, 512], F16, tag="t12")
                    # DVE: two 512-wide psum + [m0|m6] adds, then fast
                    # (2x fp16) finishes — Pool only issues stores
                    nc.vector.tensor_add(t03[:], ps[:, 0:512], msb[:, :, 0, :])
                    nc.vector.tensor_add(t12[:], ps[:, 512:1024], msb[:, :, 0, :])
                    nc.vector.tensor_add(out_sb[:, 0:256], t03[:, 0:256], msb[:, 0, 1])
                    nc.vector.tensor_sub(out_sb[:, 256:512], t12[:, 0:256], msb[:, 0, 1])
                    nc.vector.tensor_add(out_sb[:, 512:768], t12[:, 256:512], msb[:, 1, 1])
                    nc.vector.tensor_sub(out_sb[:, 768:1024], t03[:, 256:512], msb[:, 1, 1])
                else:
                    # remaining evacuations ([O1|O2] already stored above):
                    # DVE and ACT in parallel, stores on separate queues
                    nc.vector.tensor_copy(out=out_sb[:, 0:256], in_=ps[:, 0:256])
                    nc.scalar.copy(out_sb[:, 768:1024], ps[:, 256:512])
                    for dst, src in pending_out:
                        nc.sync.dma_start(dst, src)
                    pending_out.clear()
                    nc.scalar.dma_start(o_d[blk, :, 0:256], out_sb[:, 0:256])
                    nc.gpsimd.dma_start(o_d[blk, :, 768:1024], out_sb[:, 768:1024])

                if not last:
                    pending_out.append((o_d[blk], out_sb[:]))
    nc.compile()
    return nc


def _pack_weights(A_stack, B_stack):
    # M_q[(sr,i),(kr,j)] = sum_r A[r,q,kr,sr] * B[r,j,i]
    M = np.einsum("rqks,rji->qsikj", A_stack, B_stack).reshape(4, 256, 256)
    N = np.stack([
        (M[0] - M[1] - M[2] + M[3]) * 0.5,    # N0
        -(M[0] + M[1] + M[2] + M[3]) * 0.5,   # N1
        (-M[0] - M[1] + M[2] + M[3]) * 0.5,   # N6
        (M[0] - M[1] + M[2] - M[3]) * 0.5,    # N7
        M[0], M[0],                            # [N2|N2]   -> [O0|O3]
        M[1], M[1],                            # [N4|N4]   -> [O1|O2]
        M[2], -M[2],                           # [N3|N3n]  -> [O0|O3]
        -M[3], M[3],                           # [N5|N5n]  -> [O1|O2]
        (M[0] + M[1] + M[2] + M[3]) * 0.5,     # N1n = -N1 (final block)
        -(M[0] - M[1] + M[2] - M[3]) * 0.5,    # N7n = -N7 (final block)
    ])  # [NT, 256, 256]
    mpack = N.reshape(NT, 2, 128, 256).transpose(2, 0, 1, 3)
    return np.ascontiguousarray(mpack.astype(np.float16))


def _pack_x(x):
    xf = np.asarray(x, dtype=np.float32).reshape(N_CORES, TOK_PER_CORE, D)
    xs = [xf[..., s * 256 : (s + 1) * 256] for s in range(4)]
    planes = np.stack(
        [
            xs[1] - xs[3], xs[1] + xs[3], xs[0] + xs[3], xs[1] - xs[2],
            xs[0] - xs[3], xs[1] + xs[2], xs[0] - xs[2], xs[0] + xs[2],
        ],
        axis=1,
    ).astype(np.float16)  # [core, 8, 1536, 256]
    # xin[core, b, p, t, k*128+n] = planes[core, t, b*128+n, k*128+p]
    pr = planes.reshape(N_CORES, 8, BLOCKS, 128, 2, 128)
    xin = pr.transpose(0, 2, 5, 1, 4, 3).reshape(N_CORES, BLOCKS, 128, 8, 256)
    return np.ascontiguousarray(xin)


def kernel(x, A_stack, B_stack, bias):
    from concourse.bass_utils import run_bass_kernel_spmd

    global _cached_nc
    A_stack = np.asarray(A_stack, dtype=np.float32)
    B_stack = np.asarray(B_stack, dtype=np.float32)
    bias = np.asarray(bias, dtype=np.float32)

    mpack = _pack_weights(A_stack, B_stack)
    xin = _pack_x(x)

    if _cached_nc is None:
        _cached_nc = _build()
    in_maps = [
        {"xin": xin[c], "mpack": mpack} for c in range(N_CORES)
    ]
    try:
        res = run_bass_kernel_spmd(
            _cached_nc, in_maps, core_ids=list(range(N_CORES)), trace=False
        )
    except Exception:
        # axon terminals occasionally throw a transient device error
        res = run_bass_kernel_spmd(
            _cached_nc, in_maps, core_ids=list(range(N_CORES)), trace=False
        )
    out = np.stack([r["out"] for r in res.results], axis=0)  # [8,12,128,1024] f16
    out = out.reshape(TOK, D).astype(np.float32) + bias
    return out.reshape(B, T, D)


# revision 4
# speedup vs baseline: 1.0167x; 1.0009x over previous
"""Trainium2 Bass kernel for nn_BalancedHamiltonLayer — quaternion rank-8.

out_n = sum_r H_r @ X_n @ B_r^T + bias collapses to a dense (1536,1024) @
(1024,1024) matmul per core whose 4x4 grid of 256x256 blocks is the left-
multiplication table of a quaternion M = (M0,M1,M2,M3):

    out_k = sum_s SGN[k][s] * x_s @ M_{k XOR s}

A rank-8 bilinear scheme computes this with 8 products p_t = c_t @ N_t
(c_t = +-1 combination of the four x sub-blocks, N_t = fixed combination of
the M_q, folded on the host) instead of 16:

    c0=x1-x3  c1=x1+x3  c2=x0+x3  c3=x1-x2  c4=x0-x3  c5=x1+x2  c6=x0-x2
    c7=x0+x2
    N0=(M0-M1-M2+M3)/2  N1=-(M0+M1+M2+M3)/2  N2=M0  N3=M2  N4=M1  N5=-M3
    N6=(-M0-M1+M2+M3)/2 N7=(M0-M1+M2-M3)/2
    O0=p0+p1+p2+p3  O1=p0-p1+p4+p5  O2=p4-p5+p6+p7  O3=p2-p3+p6-p7

All I/O is fp16 (the 2e-2 gate leaves ~20x headroom): the host ships the
eight x-combos pre-transposed, the device returns fp16, the host upcasts
and adds bias.

Per 128-token block (PSUM tile [128,2048], regions [O0|O3|O1|O2|m0|m1|m6|m7]):
  p2,p3 (resp. p4,p5) each stream ONCE as an N=512 matmul into the adjacent
  accumulator pair [O0|O3] ([O1|O2]) with per-half signs folded into the
  duplicated/negated weight pack -> their second use costs no vector work;
  p0,p1,p6,p7 materialize, ACT evacuates them pairwise ([128,512] copies)
  to fp16, DVE does the four psum+fp16 adds, Pool/DVE finish in fp16:
    out0 = (O0 + m0) + m1   out1 = (O1 + m0) - m1
    out2 = (O2 + m6) + m7   out3 = (O3 + m6) - m7
  16 matmuls/block (engine 6144 cyc).  In-DMA on the SP queue only (out on
  ACT) so prefetches never sit behind post-dependent stores.

Matmults carry at most ONE sync wait: a zero-cost standalone ldweights
absorbs each block's x-DMA wait (3 more in block 0 for the staged weight
DMAs); each PSUM region group's first matmul carries only its single WAR
wait, with all stream-target consumers on DVE and all materialized-product
consumers on ACT.
"""

import numpy as np

B, T, D = 48, 256, 1024
N_CORES = 8
TOK = B * T
TOK_PER_CORE = TOK // N_CORES   # 1536
BLOCKS = TOK_PER_CORE // 128    # 12
NT = 14                         # weight tiles incl. duplicated/negated copies

_cached_nc = None


def _build():
    import concourse.bacc as bacc
    import concourse.mybir as mybir
    import concourse.tile as tile

    F16 = mybir.dt.float16
    F32 = mybir.dt.float32

    nc = bacc.Bacc("TRN2", target_bir_lowering=False)
    x_d = nc.dram_tensor("xin", [BLOCKS, 128, 6, 256], F16, kind="ExternalInput")
    mp_d = nc.dram_tensor("mpack", [128, NT, 2, 256], F16, kind="ExternalInput")
    o_d = nc.dram_tensor("out", [BLOCKS, 128, D], F16, kind="ExternalOutput")

    with tile.TileContext(nc) as tc:
        with (
            tc.tile_pool(name="consts", bufs=1) as consts,
            tc.tile_pool(name="xin", bufs=8) as xin_pool,
            tc.tile_pool(name="msb", bufs=4) as msb_pool,
            tc.tile_pool(name="tch", bufs=4) as tch_pool,
            tc.tile_pool(name="cmb", bufs=3) as cmb_pool,
            tc.tile_pool(name="outp", bufs=5) as out_pool,
            tc.tile_pool(name="psum", bufs=2, space="PSUM") as psum_pool,
        ):
            m_sb = consts.tile([128, NT, 2, 256], F16)

            xin = {}
            pending_out = []

            def fetch(b):
                # inputs alternate between the SP and ACT hwdge queues so
                # neither serializes the whole input stream
                t = xin_pool.tile([128, 6, 256], F16, tag="xin", name=f"xin{b}")
                eng = nc.sync if b % 2 == 0 else nc.scalar
                eng.dma_start(t[:], x_d[b])
                xin[b] = t

            # issue order: W1 (materialized-product tiles) early, stream
            # tiles (W2a: [O0|O3], W2b: [O1|O2]) interleaved with the first
            # x fetches
            nc.sync.dma_start(m_sb[:, 0:4], mp_d[:, 0:4])
            fetch(0)
            nc.scalar.dma_start(m_sb[:, 4:8], mp_d[:, 4:8])
            nc.scalar.dma_start(m_sb[:, 8:12], mp_d[:, 8:12])
            fetch(1)
            fetch(2)
            fetch(3)

            # warm-up: keep the PE continuously busy through the prologue
            # DMAs so the p-state ramp completes before real matmuls start
            wtile = consts.tile([128, 128], F16)
            nc.vector.memset(wtile[:], 0.25)
            ps_warm = psum_pool.tile([128, 2048], F32, tag="ps", name="ps_warm")
            for _w in range(40):
                nc.tensor.matmul(
                    ps_warm[:, 0:128], wtile[:], wtile[:],
                    start=True, stop=True, skip_group_check=True,
                )

            for blk in range(BLOCKS):
                if blk + 4 < BLOCKS:
                    fetch(blk + 4)
                if blk == 5:
                    # tiles for the all-stream final block ([N1|N1n],
                    # [N7|N7n] second halves) — needed only at block 11
                    nc.scalar.dma_start(m_sb[:, 12:14], mp_d[:, 12:14])
                xb = xin.pop(blk)

                ps = psum_pool.tile([128, 2048], F32, tag="ps")
                O0, O3 = ps[:, 0:256], ps[:, 256:512]
                O1, O2 = ps[:, 512:768], ps[:, 768:1024]
                m01 = ps[:, 1024:1536]
                m67 = ps[:, 1536:2048]

                # on-device difference combos from the shipped sums, all
                # on DVE (GPSIMD has no scalar_tensor_tensor):
                # c0=s13-2*x3, c3=s12-2*x2, c4=s03-2*x3, c6=s02-2*x2
                from concourse.alu_op_type import AluOpType as AO
                cmb = cmb_pool.tile([128, 4, 256], F16, tag="cmb")
                nc.vector.scalar_tensor_tensor(cmb[:, 2], xb[:, 1], -2.0, xb[:, 2], AO.mult, AO.add)
                nc.vector.scalar_tensor_tensor(cmb[:, 0], xb[:, 1], -2.0, xb[:, 3], AO.mult, AO.add)
                nc.vector.scalar_tensor_tensor(cmb[:, 1], xb[:, 0], -2.0, xb[:, 4], AO.mult, AO.add)
                nc.vector.scalar_tensor_tensor(cmb[:, 3], xb[:, 0], -2.0, xb[:, 5], AO.mult, AO.add)

                # stationary operands: planes [x2,x3,s03,s13,s12,s02]
                STAT = {
                    0: (cmb, 0), 1: (xb, 3), 2: (xb, 2), 3: (cmb, 1),
                    4: (cmb, 2), 5: (xb, 4), 6: (cmb, 3), 7: (xb, 5),
                }

                # absorb the x-DMA wait on a zero-cost weight load so every
                # real Matmult below carries at most one (WAR) wait.  In
                # block 0 the three weight-stage absorbers are interleaved
                # right before the first matmul of each stage (an absorber
                # placed earlier would gate the whole in-order PE queue).
                nc.tensor.ldweights(xb[:, 0, 0:128])
                if blk == 0:
                    nc.tensor.ldweights(m_sb[:, 0, 0, 0:128])

                def stat_ap(prod, k):
                    t, j = STAT[prod]
                    return t[:, j, k * 128 : (k + 1) * 128]

                def mm(dst, prod, tslice, start, stop):
                    for k in range(2):
                        nc.tensor.matmul(
                            dst,
                            stat_ap(prod, k),
                            m_sb[:, tslice, k, :],
                            start=start and k == 0,
                            stop=stop and k == 1,
                            skip_group_check=True,
                        )

                last = blk == BLOCKS - 1
                out_sb = out_pool.tile([128, D], F16, tag="out_sb")
                if not last:
                    # materialized products (N=256); ldweights absorbers
                    # carry the combo (DVE/Pool) waits
                    nc.tensor.ldweights(cmb[:, 0, 0:128])
                    mm(ps[:, 1024:1280], 0, 0, True, True)   # p0 = c0 @ N0
                    mm(ps[:, 1280:1536], 1, 1, True, True)   # p1 = c1 @ N1
                    nc.tensor.ldweights(cmb[:, 3, 0:128])
                    mm(ps[:, 1536:1792], 6, 2, True, True)   # p6 = c6 @ N6
                    mm(ps[:, 1792:2048], 7, 3, True, True)   # p7 = c7 @ N7
                    # dual-destination streams (N=512): [O0|O3] and [O1|O2]
                    if blk == 0:
                        for _w in range(8):
                            nc.tensor.matmul(ps_warm[:, 0:128], wtile[:], wtile[:], start=True, stop=True, skip_group_check=True)
                        nc.tensor.ldweights(m_sb[:, 4, 0, 0:128])
                    mm(ps[:, 0:512], 2, slice(4, 6), True, False)    # [+p2|+p2]
                    mm(ps[:, 512:1024], 4, slice(6, 8), True, False)   # [+p4|+p4]
                    if blk == 0:
                        for _w in range(6):
                            nc.tensor.matmul(ps_warm[:, 0:128], wtile[:], wtile[:], start=True, stop=True, skip_group_check=True)
                        nc.tensor.ldweights(m_sb[:, 8, 0, 0:128])
                    mm(ps[:, 0:512], 3, slice(8, 10), False, True)   # [+p3|-p3]
                    mm(ps[:, 512:1024], 5, slice(10, 12), False, True)  # [+p5|-p5]
                else:
                    # ALL-STREAM final block: every product accumulates
                    # directly in PSUM (no vector chain at the tail), and
                    # the [O1|O2] half completes first so its evacuation +
                    # store overlap the [O0|O3] matmuls
                    nc.tensor.ldweights(cmb[:, 2, 0:128])
                    mm(ps[:, 512:1024], 4, slice(6, 8), True, False)     # [+p4|+p4]
                    mm(ps[:, 512:1024], 5, slice(10, 12), False, False)  # [+p5|-p5]
                    nc.tensor.ldweights(cmb[:, 0, 0:128])
                    mm(ps[:, 512:768], 0, 0, False, False)    # +p0 -> O1
                    mm(ps[:, 512:768], 1, 12, False, False)   # -p1 -> O1 (N1n)
                    nc.tensor.ldweights(cmb[:, 3, 0:128])
                    mm(ps[:, 768:1024], 6, 2, False, False)   # +p6 -> O2
                    mm(ps[:, 768:1024], 7, 3, False, True)    # +p7 -> O2
                    nc.scalar.copy(out_sb[:, 256:768], ps[:, 512:1024])
                    nc.gpsimd.dma_start(o_d[blk, :, 256:768], out_sb[:, 256:768])
                    mm(ps[:, 0:512], 2, slice(4, 6), True, False)      # [+p2|+p2]
                    mm(ps[:, 0:512], 3, slice(8, 10), False, False)    # [+p3|-p3]
                    mm(ps[:, 0:256], 0, 0, False, False)      # +p0 -> O0
                    mm(ps[:, 0:256], 1, 1, False, False)      # +p1 -> O0
                    mm(ps[:, 256:512], 6, 2, False, False)    # +p6 -> O3
                    mm(ps[:, 256:512], 7, 13, False, True)    # -p7 -> O3 (N7n)

                if not last:
                    msb = msb_pool.tile([128, 2, 2, 256], F16, tag="msb")
                    nc.scalar.copy(msb[:, 0], m01)
                    nc.scalar.copy(msb[:, 1], m67)

                # issue a TWO-block-old store now (after this block's
                # evacs): its producers are long done, so the DMA carries
                # no blocking wait; Pool (SWDGE) has no other engine work
                if len(pending_out) >= 2:
                    dst, src = pending_out.pop(0)
                    nc.gpsimd.dma_start(dst, src)

                if not last:
                    t03 = tch_pool.tile([128, 512], F16, tag="t03")
                    t12 = tch_pool.tile([128, 512], F16, tag="t12")
                    # DVE: two 512-wide psum + [m0|m6] adds, then fast
                    # (2x fp16) finishes — Pool only issues stores
                    nc.vector.tensor_add(t03[:], ps[:, 0:512], msb[:, :, 0, :])
                    nc.vector.tensor_add(t12[:], ps[:, 512:1024], msb[:, :, 0, :])
                    nc.vector.tensor_add(out_sb[:, 0:256], t03[:, 0:256], msb[:, 0, 1])
                    nc.gpsimd.tensor_sub(out_sb[:, 256:512], t12[:, 0:256], msb[:, 0, 1])
                    nc.vector.tensor_add(out_sb[:, 512:768], t12[:, 256:512], msb[:, 1, 1])
                    nc.gpsimd.tensor_sub(out_sb[:, 768:1024], t03[:, 256:512], msb[:, 1, 1])
                else:
                    # remaining evacuations ([O1|O2] already stored above):
                    # DVE and ACT in parallel, stores on separate queues
                    nc.vector.tensor_copy(out=out_sb[:, 0:256], in_=ps[:, 0:256])
                    nc.scalar.copy(out_sb[:, 768:1024], ps[:, 256:512])
                    for dst, src in pending_out:
                        nc.sync.dma_start(dst, src)
                    pending_out.clear()
                    nc.scalar.dma_start(o_d[blk, :, 0:256], out_sb[:, 0:256])
                    nc.gpsimd.dma_start(o_d[blk, :, 768:1024], out_sb[:, 768:1024])

                if not last:
                    pending_out.append((o_d[blk], out_sb[:]))
    nc.compile()
    return nc


def _pack_weights(A_stack, B_stack):
    # M_q[(sr,i),(kr,j)] = sum_r A[r,q,kr,sr] * B[r,j,i]
    M = np.einsum("rqks,rji->qsikj", A_stack, B_stack).reshape(4, 256, 256)
    N = np.stack([
        (M[0] - M[1] - M[2] + M[3]) * 0.5,    # N0
        -(M[0] + M[1] + M[2] + M[3]) * 0.5,   # N1
        (-M[0] - M[1] + M[2] + M[3]) * 0.5,   # N6
        (M[0] - M[1] + M[2] - M[3]) * 0.5,    # N7
        M[0], M[0],                            # [N2|N2]   -> [O0|O3]
        M[1], M[1],                            # [N4|N4]   -> [O1|O2]
        M[2], -M[2],                           # [N3|N3n]  -> [O0|O3]
        -M[3], M[3],                           # [N5|N5n]  -> [O1|O2]
        (M[0] + M[1] + M[2] + M[3]) * 0.5,     # N1n = -N1 (final block)
        -(M[0] - M[1] + M[2] - M[3]) * 0.5,    # N7n = -N7 (final block)
    ])  # [NT, 256, 256]
    mpack = N.reshape(NT, 2, 128, 256).transpose(2, 0, 1, 3)
    return np.ascontiguousarray(mpack.astype(np.float16))


def _pack_x(x):
    xf = np.asarray(x, dtype=np.float32).reshape(N_CORES, TOK_PER_CORE, D)
    xs = [xf[..., s * 256 : (s + 1) * 256] for s in range(4)]
    planes = np.stack(
        [
            xs[2], xs[3], xs[0] + xs[3], xs[1] + xs[3],
            xs[1] + xs[2], xs[0] + xs[2],
        ],
        axis=1,
    ).astype(np.float16)  # [core, 6, 1536, 256]
    # xin[core, b, p, t, k*128+n] = planes[core, t, b*128+n, k*128+p]
    pr = planes.reshape(N_CORES, 6, BLOCKS, 128, 2, 128)
    xin = pr.transpose(0, 2, 5, 1, 4, 3).reshape(N_CORES, BLOCKS, 128, 6, 256)
    return np.ascontiguousarray(xin)


def kernel(x, A_stack, B_stack, bias):
    from concourse.bass_utils import run_bass_kernel_spmd

    global _cached_nc
    A_stack = np.asarray(A_stack, dtype=np.float32)
    B_stack = np.asarray(B_stack, dtype=np.float32)
    bias = np.asarray(bias, dtype=np.float32)

    mpack = _pack_weights(A_stack, B_stack)
    xin = _pack_x(x)

    if _cached_nc is None:
        _cached_nc = _build()
    in_maps = [
        {"xin": xin[c], "mpack": mpack} for c in range(N_CORES)
    ]
    try:
        res = run_bass_kernel_spmd(
            _cached_nc, in_maps, core_ids=list(range(N_CORES)), trace=False
        )
    except Exception:
        # axon terminals occasionally throw a transient device error
        res = run_bass_kernel_spmd(
            _cached_nc, in_maps, core_ids=list(range(N_CORES)), trace=False
        )
    out = np.stack([r["out"] for r in res.results], axis=0)  # [8,12,128,1024] f16
    out = out.reshape(TOK, D).astype(np.float32) + bias
    return out.reshape(B, T, D)


# revision 6
# speedup vs baseline: 1.0859x; 1.0681x over previous
"""Trainium2 Bass kernel for nn_BalancedHamiltonLayer — quaternion rank-8.

out_n = sum_r H_r @ X_n @ B_r^T + bias collapses to a dense (1536,1024) @
(1024,1024) matmul per core whose 4x4 grid of 256x256 blocks is the left-
multiplication table of a quaternion M = (M0,M1,M2,M3):

    out_k = sum_s SGN[k][s] * x_s @ M_{k XOR s}

A rank-8 bilinear scheme computes this with 8 products p_t = c_t @ N_t
(c_t = +-1 combination of the four x sub-blocks, N_t = fixed combination of
the M_q, folded on the host) instead of 16:

    c0=x1-x3  c1=x1+x3  c2=x0+x3  c3=x1-x2  c4=x0-x3  c5=x1+x2  c6=x0-x2
    c7=x0+x2
    N0=(M0-M1-M2+M3)/2  N1=-(M0+M1+M2+M3)/2  N2=M0  N3=M2  N4=M1  N5=-M3
    N6=(-M0-M1+M2+M3)/2 N7=(M0-M1+M2-M3)/2
    O0=p0+p1+p2+p3  O1=p0-p1+p4+p5  O2=p4-p5+p6+p7  O3=p2-p3+p6-p7

All I/O is fp16 (the 2e-2 gate leaves ~20x headroom): the host ships the
eight x-combos pre-transposed, the device returns fp16, the host upcasts
and adds bias.

Per 128-token block (PSUM tile [128,2048], regions [O0|O3|O1|O2|m0|m1|m6|m7]):
  p2,p3 (resp. p4,p5) each stream ONCE as an N=512 matmul into the adjacent
  accumulator pair [O0|O3] ([O1|O2]) with per-half signs folded into the
  duplicated/negated weight pack -> their second use costs no vector work;
  p0,p1,p6,p7 materialize, ACT evacuates them pairwise ([128,512] copies)
  to fp16, DVE does the four psum+fp16 adds, Pool/DVE finish in fp16:
    out0 = (O0 + m0) + m1   out1 = (O1 + m0) - m1
    out2 = (O2 + m6) + m7   out3 = (O3 + m6) - m7
  16 matmuls/block (engine 6144 cyc).  In-DMA on the SP queue only (out on
  ACT) so prefetches never sit behind post-dependent stores.

Matmults carry at most ONE sync wait: a zero-cost standalone ldweights
absorbs each block's x-DMA wait (3 more in block 0 for the staged weight
DMAs); each PSUM region group's first matmul carries only its single WAR
wait, with all stream-target consumers on DVE and all materialized-product
consumers on ACT.
"""

import numpy as np

B, T, D = 48, 256, 1024
N_CORES = 8
TOK = B * T
TOK_PER_CORE = TOK // N_CORES   # 1536
BLOCKS = TOK_PER_CORE // 128    # 12
NT = 14                         # weight tiles incl. duplicated/negated copies

_cached_nc = None


def _build():
    import concourse.bacc as bacc
    import concourse.mybir as mybir
    import concourse.tile as tile

    F16 = mybir.dt.float16
    F32 = mybir.dt.float32

    nc = bacc.Bacc("TRN2", target_bir_lowering=False)
    x_d = nc.dram_tensor("xin", [BLOCKS, 128, 4, 256], F16, kind="ExternalInput")
    mp_d = nc.dram_tensor("mpack", [128, NT, 2, 256], F16, kind="ExternalInput")
    o_d = nc.dram_tensor("out", [BLOCKS, 128, D], F16, kind="ExternalOutput")

    with tile.TileContext(nc) as tc:
        with (
            tc.tile_pool(name="consts", bufs=1) as consts,
            tc.tile_pool(name="xin", bufs=8) as xin_pool,
            tc.tile_pool(name="msb", bufs=4) as msb_pool,
            tc.tile_pool(name="tch", bufs=4) as tch_pool,
            tc.tile_pool(name="cmb", bufs=3) as cmb_pool,
            tc.tile_pool(name="outp", bufs=5) as out_pool,
            tc.tile_pool(name="psum", bufs=2, space="PSUM") as psum_pool,
        ):
            m_sb = consts.tile([128, NT, 2, 256], F16)

            xin = {}
            pending_out = []

            def fetch(b):
                # inputs alternate between the SP and ACT hwdge queues so
                # neither serializes the whole input stream
                t = xin_pool.tile([128, 4, 256], F16, tag="xin", name=f"xin{b}")
                eng = nc.sync if b % 2 == 0 else nc.scalar
                eng.dma_start(t[:], x_d[b])
                xin[b] = t

            # issue order: W1 (materialized-product tiles) early, stream
            # tiles (W2a: [O0|O3], W2b: [O1|O2]) interleaved with the first
            # x fetches
            nc.sync.dma_start(m_sb[:, 0:4], mp_d[:, 0:4])
            fetch(0)
            nc.scalar.dma_start(m_sb[:, 4:8], mp_d[:, 4:8])
            nc.scalar.dma_start(m_sb[:, 8:12], mp_d[:, 8:12])
            fetch(1)
            fetch(2)
            fetch(3)

            # warm-up: keep the PE continuously busy through the prologue
            # DMAs so the p-state ramp completes before real matmuls start
            wtile = consts.tile([128, 128], F16)
            nc.vector.memset(wtile[:], 0.25)
            ps_warm = psum_pool.tile([128, 2048], F32, tag="ps", name="ps_warm")
            for _w in range(30):
                nc.tensor.matmul(
                    ps_warm[:, 0:128], wtile[:], wtile[:],
                    start=True, stop=True, skip_group_check=True,
                )

            for blk in range(BLOCKS):
                if blk + 4 < BLOCKS:
                    fetch(blk + 4)
                if blk == 5:
                    # tiles for the all-stream final block ([N1|N1n],
                    # [N7|N7n] second halves) — needed only at block 11
                    nc.scalar.dma_start(m_sb[:, 12:14], mp_d[:, 12:14])
                xb = xin.pop(blk)
                if blk == BLOCKS - 1:
                    # drain pending stores now — their producers finished
                    # during earlier blocks, so SP runs them concurrently
                    # with this block's matmuls
                    for dst, src in pending_out:
                        nc.sync.dma_start(dst, src)
                    pending_out.clear()

                ps = psum_pool.tile([128, 2048], F32, tag="ps")
                O0, O3 = ps[:, 0:256], ps[:, 256:512]
                O1, O2 = ps[:, 512:768], ps[:, 768:1024]
                m01 = ps[:, 1024:1536]
                m67 = ps[:, 1536:2048]

                # build ALL eight combos on-device from the four raw
                # transposed x planes (fp16 adds/subs run at DVE 2x).  The
                # DVE emission order (c0,c7,c1,c2,c4,c5) is chosen so each
                # product's first matmul needs at most one wait: a later
                # combo's semaphore value subsumes all earlier ones.
                cmb = cmb_pool.tile([128, 8, 256], F16, tag="cmb")
                nc.vector.tensor_sub(cmb[:, 0], xb[:, 1], xb[:, 3])   # c0=x1-x3
                nc.vector.tensor_add(cmb[:, 7], xb[:, 0], xb[:, 2])   # c7=x0+x2
                nc.vector.tensor_add(cmb[:, 1], xb[:, 1], xb[:, 3])   # c1=x1+x3
                nc.vector.tensor_add(cmb[:, 2], xb[:, 0], xb[:, 3])   # c2=x0+x3
                nc.vector.tensor_sub(cmb[:, 4], xb[:, 0], xb[:, 3])   # c4=x0-x3
                nc.vector.tensor_add(cmb[:, 5], xb[:, 1], xb[:, 2])   # c5=x1+x2
                nc.gpsimd.tensor_sub(cmb[:, 6], xb[:, 0], xb[:, 2])   # c6=x0-x2
                nc.gpsimd.tensor_sub(cmb[:, 3], xb[:, 1], xb[:, 2])   # c3=x1-x2

                STAT = {t: (cmb, t) for t in range(8)}

                # absorb the x-DMA wait on a zero-cost weight load so every
                # real Matmult below carries at most one (WAR) wait.  In
                # block 0 the three weight-stage absorbers are interleaved
                # right before the first matmul of each stage (an absorber
                # placed earlier would gate the whole in-order PE queue).
                nc.tensor.ldweights(xb[:, 0, 0:128])
                if blk == 0:
                    nc.tensor.ldweights(m_sb[:, 0, 0, 0:128])

                def stat_ap(prod, k):
                    t, j = STAT[prod]
                    return t[:, j, k * 128 : (k + 1) * 128]

                def mm(dst, prod, tslice, start, stop):
                    for k in range(2):
                        nc.tensor.matmul(
                            dst,
                            stat_ap(prod, k),
                            m_sb[:, tslice, k, :],
                            start=start and k == 0,
                            stop=stop and k == 1,
                            skip_group_check=True,
                        )

                last = blk == BLOCKS - 1
                out_sb = out_pool.tile([128, D], F16, tag="out_sb")
                if not last:
                    # materialized products (N=256); the cmb0 absorber
                    # carries the first DVE-combo wait
                    nc.tensor.ldweights(cmb[:, 0, 0:128])
                    mm(ps[:, 1024:1280], 0, 0, True, True)   # p0 = c0 @ N0
                    mm(ps[:, 1280:1536], 1, 1, True, True)   # p1 = c1 @ N1
                    mm(ps[:, 1792:2048], 7, 3, True, True)   # p7 = c7 @ N7
                    mm(ps[:, 1536:1792], 6, 2, True, True)   # p6 = c6 @ N6
                    # dual-destination streams (N=512): [O0|O3] and [O1|O2]
                    if blk == 0:
                        for _w in range(4):
                            nc.tensor.matmul(ps_warm[:, 0:128], wtile[:], wtile[:], start=True, stop=True, skip_group_check=True)
                        nc.tensor.ldweights(m_sb[:, 4, 0, 0:128])
                    mm(ps[:, 0:512], 2, slice(4, 6), True, False)    # [+p2|+p2]
                    mm(ps[:, 512:1024], 4, slice(6, 8), True, False)   # [+p4|+p4]
                    if blk == 0:
                        for _w in range(6):
                            nc.tensor.matmul(ps_warm[:, 0:128], wtile[:], wtile[:], start=True, stop=True, skip_group_check=True)
                        nc.tensor.ldweights(m_sb[:, 8, 0, 0:128])
                    mm(ps[:, 0:512], 3, slice(8, 10), False, True)   # [+p3|-p3]
                    mm(ps[:, 512:1024], 5, slice(10, 12), False, True)  # [+p5|-p5]
                else:
                    # ALL-STREAM final block: every product accumulates
                    # directly in PSUM (no vector chain at the tail), and
                    # the [O1|O2] half completes first so its evacuation +
                    # store overlap the [O0|O3] matmuls
                    mm(ps[:, 512:1024], 4, slice(6, 8), True, False)     # [+p4|+p4]
                    mm(ps[:, 512:1024], 5, slice(10, 12), False, False)  # [+p5|-p5]
                    nc.tensor.ldweights(cmb[:, 0, 0:128])
                    mm(ps[:, 512:768], 0, 0, False, False)    # +p0 -> O1
                    mm(ps[:, 512:768], 1, 12, False, False)   # -p1 -> O1 (N1n)
                    nc.tensor.ldweights(cmb[:, 3, 0:128])
                    mm(ps[:, 768:1024], 6, 2, False, False)   # +p6 -> O2
                    mm(ps[:, 768:1024], 7, 3, False, True)    # +p7 -> O2
                    nc.scalar.copy(out_sb[:, 256:768], ps[:, 512:1024])
                    nc.gpsimd.dma_start(o_d[blk, :, 256:768], out_sb[:, 256:768])
                    mm(ps[:, 0:512], 2, slice(4, 6), True, False)      # [+p2|+p2]
                    mm(ps[:, 0:512], 3, slice(8, 10), False, False)    # [+p3|-p3]
                    mm(ps[:, 0:256], 0, 0, False, False)      # +p0 -> O0
                    mm(ps[:, 0:256], 1, 1, False, False)      # +p1 -> O0
                    mm(ps[:, 256:512], 6, 2, False, False)    # +p6 -> O3
                    mm(ps[:, 256:512], 7, 13, False, True)    # -p7 -> O3 (N7n)

                if not last:
                    msb = msb_pool.tile([128, 2, 2, 256], F16, tag="msb")
                    nc.scalar.copy(msb[:, 0], m01)
                    nc.scalar.copy(msb[:, 1], m67)

                # issue a TWO-block-old store now (after this block's
                # evacs): its producers are long done, so the DMA carries
                # no blocking wait and cannot head-of-line block the evacs
                if len(pending_out) >= 2:
                    dst, src = pending_out.pop(0)
                    nc.sync.dma_start(dst, src)

                if not last:
                    t03 = tch_pool.tile([128, 512], F16, tag="t03")
                    t12 = tch_pool.tile([128, 512], F16, tag="t12")
                    # DVE: two 512-wide psum + [m0|m6] adds, then fast
                    # (2x fp16) finishes — Pool only issues stores
                    nc.vector.tensor_add(t03[:], ps[:, 0:512], msb[:, :, 0, :])
                    nc.vector.tensor_add(t12[:], ps[:, 512:1024], msb[:, :, 0, :])
                    nc.vector.tensor_add(out_sb[:, 0:256], t03[:, 0:256], msb[:, 0, 1])
                    nc.gpsimd.tensor_sub(out_sb[:, 256:512], t12[:, 0:256], msb[:, 0, 1])
                    nc.vector.tensor_add(out_sb[:, 512:768], t12[:, 256:512], msb[:, 1, 1])
                    nc.gpsimd.tensor_sub(out_sb[:, 768:1024], t03[:, 256:512], msb[:, 1, 1])
                else:
                    # remaining evacuations ([O1|O2] already stored above):
                    # DVE and ACT in parallel, stores on separate queues
                    nc.vector.tensor_copy(out=out_sb[:, 0:256], in_=ps[:, 0:256])
                    nc.scalar.copy(out_sb[:, 768:1024], ps[:, 256:512])
                    nc.scalar.dma_start(o_d[blk, :, 0:256], out_sb[:, 0:256])
                    nc.gpsimd.dma_start(o_d[blk, :, 768:1024], out_sb[:, 768:1024])

                if not last:
                    pending_out.append((o_d[blk], out_sb[:]))
    nc.compile()
    return nc


def _pack_weights(A_stack, B_stack):
    # M_q[(sr,i),(kr,j)] = sum_r A[r,q,kr,sr] * B[r,j,i]
    M = np.einsum("rqks,rji->qsikj", A_stack, B_stack).reshape(4, 256, 256)
    N = np.stack([
        (M[0] - M[1] - M[2] + M[3]) * 0.5,    # N0
        -(M[0] + M[1] + M[2] + M[3]) * 0.5,   # N1
        (-M[0] - M[1] + M[2] + M[3]) * 0.5,   # N6
        (M[0] - M[1] + M[2] - M[3]) * 0.5,    # N7
        M[0], M[0],                            # [N2|N2]   -> [O0|O3]
        M[1], M[1],                            # [N4|N4]   -> [O1|O2]
        M[2], -M[2],                           # [N3|N3n]  -> [O0|O3]
        -M[3], M[3],                           # [N5|N5n]  -> [O1|O2]
        (M[0] + M[1] + M[2] + M[3]) * 0.5,     # N1n = -N1 (final block)
        -(M[0] - M[1] + M[2] - M[3]) * 0.5,    # N7n = -N7 (final block)
    ])  # [NT, 256, 256]
    mpack = N.reshape(NT, 2, 128, 256).transpose(2, 0, 1, 3)
    return np.ascontiguousarray(mpack.astype(np.float16))


def _pack_x(x):
    xf = np.asarray(x, dtype=np.float32).reshape(N_CORES, TOK_PER_CORE, D)
    xs = [xf[..., s * 256 : (s + 1) * 256] for s in range(4)]
    planes = np.stack(xs, axis=1).astype(np.float16)  # [core, 4, 1536, 256]
    # xin[core, b, p, t, k*128+n] = planes[core, t, b*128+n, k*128+p]
    pr = planes.reshape(N_CORES, 4, BLOCKS, 128, 2, 128)
    xin = pr.transpose(0, 2, 5, 1, 4, 3).reshape(N_CORES, BLOCKS, 128, 4, 256)
    return np.ascontiguousarray(xin)


def kernel(x, A_stack, B_stack, bias):
    from concourse.bass_utils import run_bass_kernel_spmd

    global _cached_nc
    A_stack = np.asarray(A_stack, dtype=np.float32)
    B_stack = np.asarray(B_stack, dtype=np.float32)
    bias = np.asarray(bias, dtype=np.float32)

    mpack = _pack_weights(A_stack, B_stack)
    xin = _pack_x(x)

    if _cached_nc is None:
        _cached_nc = _build()
    in_maps = [
        {"xin": xin[c], "mpack": mpack} for c in range(N_CORES)
    ]
    try:
        res = run_bass_kernel_spmd(
            _cached_nc, in_maps, core_ids=list(range(N_CORES)), trace=False
        )
    except Exception:
        # axon terminals occasionally throw a transient device error
        res = run_bass_kernel_spmd(
            _cached_nc, in_maps, core_ids=list(range(N_CORES)), trace=False
        )
    out = np.stack([r["out"] for r in res.results], axis=0)  # [8,12,128,1024] f16
    out = out.reshape(TOK, D).astype(np.float32) + bias
    return out.reshape(B, T, D)
